# revision 1
# baseline (speedup 1.0000x reference)
"""BasisNetwork (continuous-conv GNN) on 8 Trainium2 NeuronCores.

Per layer (nodes dest-sharded across cores, all compute in bf16/psum-f32):
  out[i] = sum_{e->i} (phi[e] (x) x[j_e]) @ Wflat  +  x[i] @ fc_w + fc_b
The per-edge basis outer product is never materialized.  Edges are host-packed
into groups of <=8 dest nodes / <=128 edge slots; a static block-diagonal
"Sphi" matrix (Sphi[k, slot*16+b] = phi[e_k, b] * [dest(e_k)==slot]) is
streamed from DRAM.  One matmul per group with the raw gathered x_j tile as
the stationary operand:
  M[c, slot*16+b] = sum_k xj[k, c] * Sphi[k, slot*16+b]
then a cheap node-side GEMM per 128 slots:
  out^T[f, slot] = sum_b conv_w[l][b].T @ M[:, slot, b]  (+ fc + bias)
with residual / relu epilogue in slot space.  Activations are all-gathered
(bf16) between layers; x_j rows are fetched with GPSIMD dma_gather from a
256B-padded table.
"""

import numpy as np

# ---------------- problem constants (hardcoded per contract) ----------------
N_NODES = 20000
F = 32               # feature width, all layers
NB = 4
B = NB * NB          # 16 basis functions
N_LAYERS = 4
OUT_SCALE = 1.0 / 128.0
NCORE = 8
NSH = N_NODES // NCORE       # 2500 dest nodes per core
GS = 8                       # node slots per group
GE = 128                     # edge slots per group
GCOL = GS * B                # 128 columns per group (slot*16+b)
EP = 128                     # padded row length (256B in bf16) for dma_gather
GCHUNK = 8                   # groups per Sphi DMA / gather chunk (1024 idxs, HW limit)
NQ = 8                       # dma_gather chunks per layer
DEBUG_STAGE = 0              # 0=full, 1=stop after scatter-GEMM, 2=no epilogue
PROFILE = False              # time warm executes
REPS = 1                     # in-NEFF repetitions of the whole network (For_i)
N_TIMED_EXECS = 8
LAST_EXEC_NS = None
LAST_TRACE = None
LAST_EXEC_TIMES = None


def _hat(x, n):
    c = np.linspace(-1.0, 1.0, n, dtype=np.float32)
    r = np.abs(x[:, None] - c[None, :]) * ((n - 1) * 0.5)
    return np.maximum(1.0 - r, 0.0).astype(np.float32)


def _edge_basis(edge_attr, edge_i, edge_j):
    d = np.clip(edge_attr.astype(np.float32), -1.0, 1.0)
    phi = (_hat(d[:, 0], NB)[:, :, None] * _hat(d[:, 1], NB)[:, None, :]).reshape(-1, B)
    phi *= (edge_i != edge_j).astype(np.float32)[:, None]
    return phi  # [E, B]


def _preprocess(edge_i, edge_j, edge_attr):
    ei = np.asarray(edge_i).astype(np.int64)
    ej = np.asarray(edge_j).astype(np.int64)
    phi = _edge_basis(np.asarray(edge_attr), ei, ej)

    deg = np.bincount(ei, minlength=N_NODES)
    packs = []
    ng_max = 0
    for c in range(NCORE):
        groups = []
        cur_nodes, cur_edges = [], 0
        for n in range(c * NSH, (c + 1) * NSH):
            d = int(deg[n])
            if cur_nodes and (len(cur_nodes) >= GS or cur_edges + d > GE):
                groups.append(cur_nodes)
                cur_nodes, cur_edges = [], 0
            cur_nodes.append(n)
            cur_edges += d
        if cur_nodes:
            groups.append(cur_nodes)
        packs.append(groups)
        ng_max = max(ng_max, len(groups))

    NG = -(-ng_max // 48) * 48      # multiple of 48 (sphi chunks, gather chunks, batches)
    NSLOT = NG * GS
    assert NCORE * NSLOT < 32768    # int16 gather indices

    slot_of_node = np.zeros(N_NODES, dtype=np.int64)
    for c in range(NCORE):
        for gi, nodes in enumerate(packs[c]):
            for s, n in enumerate(nodes):
                slot_of_node[n] = c * NSLOT + gi * GS + s

    # per-core tables
    sphi = np.zeros((NCORE, NG, GE, GCOL), dtype=np.float32)
    gsrc = np.zeros((NCORE, GE, NG), dtype=np.int16)   # [edge slot p, group] -> src row
    order_all = np.argsort(ei, kind="stable")
    starts = np.zeros(N_NODES + 1, dtype=np.int64)
    np.cumsum(deg, out=starts[1:])
    for c in range(NCORE):
        for gi, nodes in enumerate(packs[c]):
            k = 0
            for s, n in enumerate(nodes):
                for e in order_all[starts[n]:starts[n + 1]]:
                    sphi[c, gi, k, s * B:(s + 1) * B] = phi[e]
                    gsrc[c, k, gi] = slot_of_node[ej[e]]
                    k += 1
            assert k <= GE

    # wrapped gather-index layout: stream k = gi*128 + p -> [k%16, k//16], x8 cores
    idx_w = np.zeros((NCORE, 16, NG * GE // 16), dtype=np.int16)
    for c in range(NCORE):
        stream = gsrc[c].T.reshape(-1)          # k = gi*128 + p
        idx_w[c, :, :] = stream.reshape(-1, 16).T
    idx_w = np.tile(idx_w, (1, 8, 1))           # [NCORE, 128, NG*8]

    return NG, NSLOT, slot_of_node, sphi, idx_w


def _build_and_run(pre, fluidFeatures, conv_ws, fc_ws, fc_bs):
    import ml_dtypes
    import concourse.bass as bass
    import concourse.bacc as bacc
    import concourse.mybir as mybir
    import concourse.tile as tile
    from concourse.bass_utils import run_bass_kernel_spmd
    from concourse.masks import make_identity
    from concourse import library_config

    NG, NSLOT, slot_of_node, sphi, idx_w = pre
    NBATCH = NSLOT // 128
    NCHUNK = NG // GCHUNK
    NGQ = NG // NQ                    # groups per gather chunk
    NIQ = NGQ * GE                    # idxs per gather chunk
    NROWS = NCORE * NSLOT

    bf16 = mybir.dt.bfloat16
    f32 = mybir.dt.float32
    i16 = mybir.dt.int16

    x0 = np.asarray(fluidFeatures, dtype=np.float32)
    x0_slots = np.zeros((NROWS, F), dtype=np.float32)
    x0_slots[slot_of_node] = x0
    x0_pad = np.zeros((NROWS, EP), dtype=np.float32)
    x0_pad[:, :F] = x0_slots

    conv_w = np.stack([np.asarray(w, dtype=np.float32) for w in conv_ws])  # [L,B,F,F]
    fc_w = np.stack([np.asarray(w, dtype=np.float32) for w in fc_ws])      # [L,F,F]
    fc_b = np.stack([np.asarray(b, dtype=np.float32) for b in fc_bs])      # [L,F]

    # ------------------------------ build graph ------------------------------
    nc = bacc.Bacc("TRN2", target_bir_lowering=False, debug=False, num_devices=NCORE)

    d_x0pad = nc.dram_tensor("x0pad", [NROWS, EP], bf16, kind="ExternalInput")
    d_x0T = nc.dram_tensor("x0T", [F, NSLOT], bf16, kind="ExternalInput")
    d_sphi = nc.dram_tensor("sphi", [GE, NG * GCOL], bf16, kind="ExternalInput")
    d_gidx = nc.dram_tensor("gidx", [128, NG * GE // 16], i16, kind="ExternalInput")
    d_convw = nc.dram_tensor("convw", [F, N_LAYERS * B * F], bf16, kind="ExternalInput")
    d_fcw = nc.dram_tensor("fcw", [F, N_LAYERS * F], bf16, kind="ExternalInput")
    d_fcb = nc.dram_tensor("fcb", [F, N_LAYERS], f32, kind="ExternalInput")
    d_out = nc.dram_tensor("out", [NSLOT, F], f32, kind="ExternalOutput")
    d_ash = nc.dram_tensor("a_shard", [NSLOT, F], bf16, kind="Internal")
    d_afull = nc.dram_tensor("a_full", [NROWS, F], bf16, kind="Internal",
                             addr_space="Shared")
    d_apad = nc.dram_tensor("a_pad", [NROWS, EP], bf16, kind="Internal")

    with tile.TileContext(nc) as tc:
        with (
            tc.tile_pool(name="persist", bufs=1) as pp,
            tc.tile_pool(name="sphi", bufs=2) as sp,
            tc.tile_pool(name="xj", bufs=2) as xp,
            tc.tile_pool(name="work", bufs=3) as wp,
            tc.tile_pool(name="psc", bufs=3, space="PSUM") as psc,
            tc.tile_pool(name="pout", bufs=2, space="PSUM") as pout,
            tc.tile_pool(name="ptr", bufs=1, space="PSUM") as ptr,
        ):
            nc.gpsimd.load_library(library_config.mlp)

            gidx_sb = pp.tile([128, NG * GE // 16], i16)
            nc.sync.dma_start(out=gidx_sb[:], in_=d_gidx[:])
            convw_sb = pp.tile([F, N_LAYERS * B * F], bf16)
            nc.sync.dma_start(out=convw_sb[:], in_=d_convw[:])
            fcw_sb = pp.tile([F, N_LAYERS * F], bf16)
            nc.sync.dma_start(out=fcw_sb[:], in_=d_fcw[:])
            fcb_sb = pp.tile([F, N_LAYERS], f32)
            nc.sync.dma_start(out=fcb_sb[:], in_=d_fcb[:])
            id_bf = pp.tile([F, F], bf16)
            make_identity(nc, id_bf[:])
            id_f32 = pp.tile([F, F], f32)
            make_identity(nc, id_f32[:])
            m_all = pp.tile([F, NSLOT * B], bf16)     # [c, slot*16+b]
            ansT = pp.tile([F, NSLOT], f32)
            aT = pp.tile([F, NSLOT], bf16)
            nc.sync.dma_start(out=aT[:], in_=d_x0T[:])

            for l in range(N_LAYERS):
                if l > 0:
                    nc.gpsimd.collective_compute(
                        "AllGather", mybir.AluOpType.bypass,
                        replica_groups=[list(range(NCORE))],
                        ins=[d_ash[:]], outs=[d_afull[:]])
                    nc.sync.dma_start(out=d_apad[:, 0:F], in_=d_afull[:])
                src = d_x0pad if l == 0 else d_apad

                # ---------- gather + scatter-GEMM, chunked by GCHUNK groups ----------
                NIC = GCHUNK * GE             # gather idxs per chunk
                for ch in range(NCHUNK):
                    xq = xp.tile([GE, GCHUNK * EP], bf16, tag="xj")
                    nc.gpsimd.dma_gather(
                        out_ap=xq[:].rearrange("p (g e) -> p g e", e=EP),
                        in_ap=src[:],
                        idxs_ap=gidx_sb[:, ch * NIC // 16:(ch + 1) * NIC // 16],
                        num_idxs=NIC, num_idxs_reg=NIC, elem_size=EP)
                    st = sp.tile([GE, GCHUNK * GCOL], bf16, tag="st")
                    nc.sync.dma_start(
                        out=st[:],
                        in_=d_sphi[:, ch * GCHUNK * GCOL:(ch + 1) * GCHUNK * GCOL])
                    for g4 in range(GCHUNK // 4):
                        ps = psc.tile([F, 4 * GCOL], f32, tag="psc")
                        for j in range(4):
                            gl = g4 * 4 + j
                            nc.tensor.matmul(
                                out=ps[:, j * GCOL:(j + 1) * GCOL],
                                lhsT=xq[:, gl * EP:gl * EP + F],
                                rhs=st[:, gl * GCOL:(gl + 1) * GCOL],
                                start=True, stop=True)
                        gi0 = ch * GCHUNK + g4 * 4
                        if g4 % 2 == 0:
                            nc.scalar.copy(
                                out=m_all[:, gi0 * GCOL:(gi0 + 4) * GCOL], in_=ps[:])
                        else:
                            nc.vector.tensor_copy(
                                out=m_all[:, gi0 * GCOL:(gi0 + 4) * GCOL], in_=ps[:])

                # ---------- node-side GEMM + epilogue per 128 slots ----------
                if DEBUG_STAGE == 1:
                    continue
                m_view = m_all[:].rearrange("c (s b) -> c b s", b=B)
                for bi in range(NBATCH):
                    sl = slice(bi * 128, (bi + 1) * 128)
                    po = pout.tile([F, 128], f32, tag="pout")
                    for b in range(B):
                        wofs = (l * B + b) * F
                        nc.tensor.matmul(
                            out=po[:], lhsT=convw_sb[:, wofs:wofs + F],
                            rhs=m_view[:, b, sl], start=(b == 0), stop=False)
                    nc.tensor.matmul(
                        out=po[:], lhsT=fcw_sb[:, l * F:(l + 1) * F],
                        rhs=aT[:, sl], start=False, stop=True)

                    if DEBUG_STAGE == 2:
                        continue
                    if l == 0:
                        nc.scalar.activation(
                            out=ansT[:, sl], in_=po[:],
                            func=mybir.ActivationFunctionType.Identity,
                            bias=fcb_sb[:, l:l + 1])
                    else:
                        tmp = wp.tile([F, 128], f32, tag="tmp")
                        nc.scalar.activation(
                            out=tmp[:], in_=po[:],
                            func=mybir.ActivationFunctionType.Identity,
                            bias=fcb_sb[:, l:l + 1])
                        nc.vector.tensor_add(
                            out=ansT[:, sl], in0=ansT[:, sl], in1=tmp[:])

                    if l < N_LAYERS - 1:
                        nc.scalar.activation(
                            out=aT[:, sl], in_=ansT[:, sl],
                            func=mybir.ActivationFunctionType.Relu)
                        pt = ptr.tile([128, F], bf16, tag="ptrb")
                        nc.tensor.transpose(out=pt[:], in_=aT[:, sl], identity=id_bf[:])
                        aout = wp.tile([128, F], bf16, tag="aout")
                        nc.vector.tensor_copy(out=aout[:], in_=pt[:])
                        nc.sync.dma_start(out=d_ash[sl, :], in_=aout[:])
                    else:
                        oT = wp.tile([F, 128], f32, tag="oT")
                        nc.scalar.activation(
                            out=oT[:], in_=ansT[:, sl],
                            func=mybir.ActivationFunctionType.Copy,
                            scale=OUT_SCALE)
                        pt = ptr.tile([128, F], f32, tag="ptr")
                        nc.tensor.transpose(out=pt[:], in_=oT[:], identity=id_f32[:])
                        oout = wp.tile([128, F], f32, tag="oout")
                        nc.vector.tensor_copy(out=oout[:], in_=pt[:])
                        nc.sync.dma_start(out=d_out[sl, :], in_=oout[:])

    nc.compile()

    # ------------------------------ run ------------------------------
    convw_c = np.ascontiguousarray(
        conv_w.transpose(2, 0, 1, 3).reshape(F, N_LAYERS * B * F)
    ).astype(ml_dtypes.bfloat16)
    fcw_c = np.ascontiguousarray(
        fc_w.transpose(1, 0, 2).reshape(F, N_LAYERS * F)).astype(ml_dtypes.bfloat16)
    fcb_T = np.ascontiguousarray(fc_b.T)
    x0_pad_bf = x0_pad.astype(ml_dtypes.bfloat16)
    sphi_bf = sphi.astype(ml_dtypes.bfloat16)

    in_maps = []
    for c in range(NCORE):
        x0T_c = np.ascontiguousarray(
            x0_slots[c * NSLOT:(c + 1) * NSLOT].T).astype(ml_dtypes.bfloat16)
        sphi_c = np.ascontiguousarray(
            sphi_bf[c].transpose(1, 0, 2).reshape(GE, NG * GCOL))
        in_maps.append({
            "x0pad": x0_pad_bf,
            "x0T": x0T_c,
            "sphi": sphi_c,
            "gidx": np.ascontiguousarray(idx_w[c]),
            "convw": convw_c,
            "fcw": fcw_c,
            "fcb": fcb_T,
        })

    res = run_bass_kernel_spmd(nc, in_maps, core_ids=list(range(NCORE)),
                               trace=PROFILE)
    global LAST_EXEC_NS, LAST_TRACE
    LAST_EXEC_NS = res.exec_time_ns
    LAST_TRACE = res.instructions_and_trace[1] if res.instructions_and_trace else None
    out_slots = np.concatenate([res.results[c]["out"] for c in range(NCORE)], axis=0)
    return out_slots[slot_of_node].astype(np.float32)


def kernel(fluidFeatures, edge_i, edge_j, edge_attr, conv_ws, fc_ws, fc_bs):
    pre = _preprocess(edge_i, edge_j, edge_attr)
    return _build_and_run(pre, fluidFeatures, conv_ws, fc_ws, fc_bs)



# revision 21
# speedup vs baseline: 2.1024x; 2.1024x over previous
"""BasisNetwork (continuous-conv GNN) on 8 Trainium2 NeuronCores.

Per layer (nodes dest-sharded across cores, all compute in bf16/psum-f32):
  out[i] = sum_{e->i} (phi[e] (x) x[j_e]) @ Wflat  +  x[i] @ fc_w + fc_b
The per-edge basis outer product is never materialized.  Edges are host-packed
into groups of <=8 dest nodes / <=128 edge slots; a static block-diagonal
"Sphi" matrix (Sphi[k, slot*16+b] = phi[e_k, b] * [dest(e_k)==slot]) is
streamed from DRAM.  One matmul per group with the raw gathered x_j tile as
the stationary operand:
  M[c, slot*16+b] = sum_k xj[k, c] * Sphi[k, slot*16+b]
then a cheap node-side GEMM per 128 slots:
  out^T[f, slot] = sum_b conv_w[l][b].T @ M[:, slot, b]  (+ fc + bias)
with residual / relu epilogue in slot space.  Activations are all-gathered
(bf16) between layers; x_j rows are fetched with GPSIMD dma_gather from a
256B-padded table.
"""

import numpy as np

# ---------------- problem constants (hardcoded per contract) ----------------
N_NODES = 20000
F = 32               # feature width, all layers
NB = 4
B = NB * NB          # 16 basis functions
N_LAYERS = 4
OUT_SCALE = 1.0 / 128.0
NCORE = 8
NSH = N_NODES // NCORE       # 2500 dest nodes per core
GS = 8                       # node slots per group
GE = 128                     # edge slots per group
GCOL = GS * B                # 128 columns per group (slot*16+b)
EP = 128                     # padded row length (256B in bf16) for dma_gather
GCHUNK = 8                   # groups per Sphi DMA / gather chunk (1024 idxs, HW limit)
NQ = 8                       # dma_gather chunks per layer
DEBUG_STAGE = 0              # 0=full, 1=stop after scatter-GEMM, 2=no epilogue
PROFILE = False              # time warm executes
REPS = 1                     # in-NEFF repetitions of the whole network (For_i)
N_TIMED_EXECS = 8
LAST_EXEC_NS = None
LAST_TRACE = None
LAST_EXEC_TIMES = None


def _hat(x, n):
    c = np.linspace(-1.0, 1.0, n, dtype=np.float32)
    r = np.abs(x[:, None] - c[None, :]) * ((n - 1) * 0.5)
    return np.maximum(1.0 - r, 0.0).astype(np.float32)


def _edge_basis(edge_attr, edge_i, edge_j):
    d = np.clip(edge_attr.astype(np.float32), -1.0, 1.0)
    phi = (_hat(d[:, 0], NB)[:, :, None] * _hat(d[:, 1], NB)[:, None, :]).reshape(-1, B)
    phi *= (edge_i != edge_j).astype(np.float32)[:, None]
    return phi  # [E, B]


def _pack_groups_ffd(nodes, degs):
    """First-fit-decreasing bin packing: nodes into groups with <=GS slots and
    <=GE total edges.  Returns list of groups (each a list of node ids)."""
    order = sorted(range(len(nodes)), key=lambda i: -degs[i])
    bins = []          # (slots_used, edges_used, [nodes])
    for i in order:
        d = degs[i]
        placed = False
        for b in bins:
            if b[0] < GS and b[1] + d <= GE:
                b[0] += 1
                b[1] += d
                b[2].append(nodes[i])
                placed = True
                break
        if not placed:
            bins.append([1, d, [nodes[i]]])
    return [b[2] for b in bins]


def _preprocess(edge_i, edge_j, edge_attr):
    ei = np.asarray(edge_i).astype(np.int64)
    ej = np.asarray(edge_j).astype(np.int64)
    phi = _edge_basis(np.asarray(edge_attr), ei, ej)

    deg = np.bincount(ei, minlength=N_NODES)
    packs = []
    ng_max = 0
    for c in range(NCORE):
        nodes = list(range(c * NSH, (c + 1) * NSH))
        degs = [int(deg[n]) for n in nodes]
        groups = _pack_groups_ffd(nodes, degs)
        packs.append(groups)
        ng_max = max(ng_max, len(groups))

    NG = -(-ng_max // 16) * 16      # multiple of 16 (sphi chunks, gather chunks, batches)
    NSLOT = NG * GS
    assert NCORE * NSLOT < 32768    # int16 gather indices

    # node-GEMM batches (<=512 slots each) and the 2-way allgather split point
    widths = []
    rem = NSLOT
    while rem > 512:
        widths.append(512)
        rem -= 512
    widths.append(rem)
    nb_half = (len(widths) + 1) // 2
    P0 = sum(widths[:nb_half])      # per-core slot split for the two collectives
    parts = [(0, P0), (P0, NSLOT)]

    # table (gather-source) row numbering: part-major, then core, then slot
    cat2tab = np.zeros(NCORE * NSLOT, dtype=np.int64)
    base = 0
    for (s0, s1) in parts:
        ln = s1 - s0
        for c in range(NCORE):
            cat = c * NSLOT + s0
            cat2tab[cat:cat + ln] = base + c * ln + np.arange(ln)
        base += NCORE * ln

    slot_of_node = np.zeros(N_NODES, dtype=np.int64)
    for c in range(NCORE):
        for gi, nodes in enumerate(packs[c]):
            for s, n in enumerate(nodes):
                slot_of_node[n] = c * NSLOT + gi * GS + s

    # per-core tables
    sphi = np.zeros((NCORE, NG, GE, GCOL), dtype=np.float32)
    gsrc = np.zeros((NCORE, GE, NG), dtype=np.int16)   # [edge slot p, group] -> src row
    order_all = np.argsort(ei, kind="stable")
    starts = np.zeros(N_NODES + 1, dtype=np.int64)
    np.cumsum(deg, out=starts[1:])
    for c in range(NCORE):
        for gi, nodes in enumerate(packs[c]):
            k = 0
            for s, n in enumerate(nodes):
                for e in order_all[starts[n]:starts[n + 1]]:
                    sphi[c, gi, k, s * B:(s + 1) * B] = phi[e]
                    gsrc[c, k, gi] = slot_of_node[ej[e]]
                    k += 1
            assert k <= GE

    # wrapped gather-index layout: stream k = gi*128 + p -> [k%16, k//16], x8 cores
    # (indices in TABLE row order; gsrc itself stays in cat order for xq0)
    idx_w = np.zeros((NCORE, 16, NG * GE // 16), dtype=np.int16)
    for c in range(NCORE):
        stream = cat2tab[gsrc[c].T.reshape(-1).astype(np.int64)]  # k = gi*128 + p
        idx_w[c, :, :] = stream.astype(np.int16).reshape(-1, 16).T
    idx_w = np.tile(idx_w, (1, 8, 1))           # [NCORE, 128, NG*8]

    return NG, NSLOT, slot_of_node, sphi, idx_w, gsrc, widths, parts


def _build_and_run(pre, fluidFeatures, conv_ws, fc_ws, fc_bs):
    import ml_dtypes
    import concourse.bass as bass
    import concourse.bacc as bacc
    import concourse.mybir as mybir
    import concourse.tile as tile
    from concourse.bass_utils import run_bass_kernel_spmd
    from concourse.masks import make_identity
    from concourse import library_config

    NG, NSLOT, slot_of_node, sphi, idx_w, gsrc, widths, parts = pre
    NCHUNK = NG // GCHUNK
    NROWS = NCORE * NSLOT
    # per-batch slot offsets
    boffs = [0]
    for w in widths:
        boffs.append(boffs[-1] + w)
    # table row ranges per collective part
    tab_ranges = []
    base = 0
    for (s0, s1) in parts:
        ln = s1 - s0
        tab_ranges.append((base, base + NCORE * ln))
        base += NCORE * ln

    bf16 = mybir.dt.bfloat16
    f32 = mybir.dt.float32
    i16 = mybir.dt.int16

    x0 = np.asarray(fluidFeatures, dtype=np.float32)
    x0_slots = np.zeros((NROWS, F), dtype=np.float32)
    x0_slots[slot_of_node] = x0

    conv_w = np.stack([np.asarray(w, dtype=np.float32) for w in conv_ws])  # [L,B,F,F]
    fc_w = np.stack([np.asarray(w, dtype=np.float32) for w in fc_ws])      # [L,F,F]
    fc_b = np.stack([np.asarray(b, dtype=np.float32) for b in fc_bs])      # [L,F]

    # ------------------------------ build graph ------------------------------
    nc = bacc.Bacc("TRN2", target_bir_lowering=False, debug=False, num_devices=NCORE,
                   num_swdge_queues=4)

    d_xq0 = nc.dram_tensor("xq0", [GE, NG * F], bf16, kind="ExternalInput")
    d_x0T = nc.dram_tensor("x0T", [F, NSLOT], bf16, kind="ExternalInput")
    d_sphi = nc.dram_tensor("sphi", [GE, NG * GCOL], bf16, kind="ExternalInput")
    d_gidx = nc.dram_tensor("gidx", [128, NG * GE // 16], i16, kind="ExternalInput")
    d_convw = nc.dram_tensor("convw", [F, N_LAYERS * B * F], bf16, kind="ExternalInput")
    d_fcw = nc.dram_tensor("fcw", [F, N_LAYERS * F], bf16, kind="ExternalInput")
    d_fcb = nc.dram_tensor("fcb", [F, N_LAYERS], f32, kind="ExternalInput")
    d_out = nc.dram_tensor("out", [NSLOT, F], f32, kind="ExternalOutput")
    d_ash = [nc.dram_tensor(f"a_shard{h}", [s1 - s0, F], bf16, kind="Internal")
             for h, (s0, s1) in enumerate(parts)]
    d_afull = [nc.dram_tensor(f"a_full{h}", [NCORE * (s1 - s0), F], bf16,
                              kind="Internal", addr_space="Shared")
               for h, (s0, s1) in enumerate(parts)]
    d_apad = nc.dram_tensor("a_pad", [NROWS, EP], bf16, kind="Internal")
    d_wu_in = nc.dram_tensor("wu_in", [128, 16], bf16, kind="Internal")
    d_wu_out = nc.dram_tensor("wu_out", [NCORE * 128, 16], bf16, kind="Internal",
                              addr_space="Shared")

    with tile.TileContext(nc) as tc:
        with (
            tc.tile_pool(name="persist", bufs=1) as pp,
            tc.tile_pool(name="sphi", bufs=3) as sp,
            tc.tile_pool(name="xj", bufs=6) as xp,
            tc.tile_pool(name="work", bufs=3) as wp,
            tc.tile_pool(name="psc", bufs=3, space="PSUM") as psc,
            tc.tile_pool(name="pout", bufs=2, space="PSUM") as pout,
            tc.tile_pool(name="ptr", bufs=1, space="PSUM") as ptr,
        ):
            nc.gpsimd.load_library(library_config.mlp)

            # tiny warmup collective: absorbs first-collective link setup cost
            # while layer-0 compute runs
            nc.gpsimd.collective_compute(
                "AllGather", mybir.AluOpType.bypass,
                replica_groups=[list(range(NCORE))],
                ins=[d_wu_in[:]], outs=[d_wu_out[:]])

            gidx_sb = pp.tile([128, NG * GE // 16], i16)
            nc.sync.dma_start(out=gidx_sb[:], in_=d_gidx[:])
            convw_sb = pp.tile([F, N_LAYERS * B * F], bf16)
            nc.sync.dma_start(out=convw_sb[:], in_=d_convw[:])
            fcw_sb = pp.tile([F, N_LAYERS * F], bf16)
            nc.sync.dma_start(out=fcw_sb[:], in_=d_fcw[:])
            fcb_sb = pp.tile([F, N_LAYERS], f32)
            nc.sync.dma_start(out=fcb_sb[:], in_=d_fcb[:])
            id_bf = pp.tile([F, F], bf16)
            make_identity(nc, id_bf[:])
            id_f32 = pp.tile([F, F], f32)
            make_identity(nc, id_f32[:])
            m_all = pp.tile([F, NSLOT * B], bf16)     # [c, slot*16+b]
            ansT = pp.tile([F, NSLOT], f32)
            aT = pp.tile([F, NSLOT], bf16)
            nc.sync.dma_start(out=aT[:], in_=d_x0T[:])
            xq0_sb = pp.tile([GE, NG * F], bf16)      # layer-0 x_j, host pre-gathered
            nc.sync.dma_start(out=xq0_sb[:], in_=d_xq0[:])

            for l in range(N_LAYERS):
                if l > 0:
                    for h, (t0, t1) in enumerate(tab_ranges):
                        nc.gpsimd.collective_compute(
                            "AllGather", mybir.AluOpType.bypass,
                            replica_groups=[list(range(NCORE))],
                            ins=[d_ash[h][:]], outs=[d_afull[h][:]])
                        nc.sync.dma_start(out=d_apad[t0:t1, 0:F],
                                          in_=d_afull[h][:])

                # ---------- gather + scatter-GEMM, chunked by GCHUNK groups ----------
                NIC = GCHUNK * GE             # gather idxs per chunk
                for ch in range(NCHUNK):
                    if l == 0:
                        xq = xq0_sb
                        xofs, xstride = ch * GCHUNK * F, F
                    else:
                        xq = xp.tile([GE, GCHUNK * EP], bf16, tag="xj")
                        xofs, xstride = 0, EP
                        nc.gpsimd.dma_gather(
                            out_ap=xq[:].rearrange("p (g e) -> p g e", e=EP),
                            in_ap=d_apad[:],
                            idxs_ap=gidx_sb[:, ch * NIC // 16:(ch + 1) * NIC // 16],
                            num_idxs=NIC, num_idxs_reg=NIC, elem_size=EP,
                            queue_num=ch % 4)
                    st = sp.tile([GE, GCHUNK * GCOL], bf16, tag="st")
                    nc.sync.dma_start(
                        out=st[:],
                        in_=d_sphi[:, ch * GCHUNK * GCOL:(ch + 1) * GCHUNK * GCOL])
                    for g4 in range(GCHUNK // 4):
                        ps = psc.tile([F, 4 * GCOL], f32, tag="psc")
                        for j in range(4):
                            gl = g4 * 4 + j
                            nc.tensor.matmul(
                                out=ps[:, j * GCOL:(j + 1) * GCOL],
                                lhsT=xq[:, xofs + gl * xstride:xofs + gl * xstride + F],
                                rhs=st[:, gl * GCOL:(gl + 1) * GCOL],
                                start=True, stop=True)
                        gi0 = ch * GCHUNK + g4 * 4
                        if g4 % 2 == 0:
                            nc.scalar.copy(
                                out=m_all[:, gi0 * GCOL:(gi0 + 4) * GCOL], in_=ps[:])
                        else:
                            nc.vector.tensor_copy(
                                out=m_all[:, gi0 * GCOL:(gi0 + 4) * GCOL], in_=ps[:])

                # ---------- node-side GEMM + epilogue per <=512 slots ----------
                if DEBUG_STAGE == 1:
                    continue
                m_view = m_all[:].rearrange("c (s b) -> c b s", b=B)
                for bi, w in enumerate(widths):
                    sl = slice(boffs[bi], boffs[bi] + w)
                    nk = w // 128
                    po = pout.tile([F, w], f32, tag="pout")
                    for b in range(B):
                        wofs = (l * B + b) * F
                        nc.tensor.matmul(
                            out=po[:], lhsT=convw_sb[:, wofs:wofs + F],
                            rhs=m_view[:, b, sl], start=(b == 0), stop=False)
                    nc.tensor.matmul(
                        out=po[:], lhsT=fcw_sb[:, l * F:(l + 1) * F],
                        rhs=aT[:, sl], start=False, stop=True)

                    if DEBUG_STAGE == 2:
                        continue
                    if l == 0:
                        nc.scalar.activation(
                            out=ansT[:, sl], in_=po[:],
                            func=mybir.ActivationFunctionType.Identity,
                            bias=fcb_sb[:, l:l + 1])
                    else:
                        tmp = wp.tile([F, w], f32, tag="tmp")
                        nc.scalar.activation(
                            out=tmp[:], in_=po[:],
                            func=mybir.ActivationFunctionType.Identity,
                            bias=fcb_sb[:, l:l + 1])
                        nc.vector.tensor_add(
                            out=ansT[:, sl], in0=ansT[:, sl], in1=tmp[:])

                    if l < N_LAYERS - 1:
                        nc.scalar.activation(
                            out=aT[:, sl], in_=ansT[:, sl],
                            func=mybir.ActivationFunctionType.Relu)
                        aout = wp.tile([128, nk * F], bf16, tag="aout")
                        for k in range(nk):
                            s2 = slice(boffs[bi] + k * 128, boffs[bi] + (k + 1) * 128)
                            pt = ptr.tile([128, F], bf16, tag="ptrb")
                            nc.tensor.transpose(out=pt[:], in_=aT[:, s2],
                                                identity=id_bf[:])
                            nc.vector.tensor_copy(
                                out=aout[:, k * F:(k + 1) * F], in_=pt[:])
                        h = 1 if boffs[bi] >= parts[1][0] else 0
                        r0 = boffs[bi] - parts[h][0]
                        nc.sync.dma_start(
                            out=d_ash[h][r0:r0 + w, :].rearrange(
                                "(k p) f -> p k f", p=128),
                            in_=aout[:].rearrange("p (k f) -> p k f", f=F))
                    else:
                        oT = wp.tile([F, w], f32, tag="oT")
                        nc.scalar.activation(
                            out=oT[:], in_=ansT[:, sl],
                            func=mybir.ActivationFunctionType.Copy,
                            scale=OUT_SCALE)
                        oout = wp.tile([128, nk * F], f32, tag="oout")
                        for k in range(nk):
                            pt = ptr.tile([128, F], f32, tag="ptr")
                            nc.tensor.transpose(out=pt[:], in_=oT[:, k * 128:(k + 1) * 128],
                                                identity=id_f32[:])
                            nc.vector.tensor_copy(
                                out=oout[:, k * F:(k + 1) * F], in_=pt[:])
                        nc.sync.dma_start(
                            out=d_out[sl, :].rearrange("(k p) f -> p k f", p=128),
                            in_=oout[:].rearrange("p (k f) -> p k f", f=F))

    nc.compile()

    # ------------------------------ run ------------------------------
    convw_c = np.ascontiguousarray(
        conv_w.transpose(2, 0, 1, 3).reshape(F, N_LAYERS * B * F)
    ).astype(ml_dtypes.bfloat16)
    fcw_c = np.ascontiguousarray(
        fc_w.transpose(1, 0, 2).reshape(F, N_LAYERS * F)).astype(ml_dtypes.bfloat16)
    fcb_T = np.ascontiguousarray(fc_b.T)
    sphi_bf = sphi.astype(ml_dtypes.bfloat16)
    x0_slots_bf = x0_slots.astype(ml_dtypes.bfloat16)

    in_maps = []
    for c in range(NCORE):
        x0T_c = np.ascontiguousarray(
            x0_slots[c * NSLOT:(c + 1) * NSLOT].T).astype(ml_dtypes.bfloat16)
        sphi_c = np.ascontiguousarray(
            sphi_bf[c].transpose(1, 0, 2).reshape(GE, NG * GCOL))
        # layer-0 x_j gathered on host: xq0[p, gi*F:(gi+1)*F] = x0[gsrc[c][p, gi]]
        xq0_c = np.ascontiguousarray(
            x0_slots_bf[gsrc[c].T.astype(np.int64)].transpose(1, 0, 2)
            .reshape(GE, NG * F))
        in_maps.append({
            "xq0": xq0_c,
            "x0T": x0T_c,
            "sphi": sphi_c,
            "gidx": np.ascontiguousarray(idx_w[c]),
            "convw": convw_c,
            "fcw": fcw_c,
            "fcb": fcb_T,
        })

    res = run_bass_kernel_spmd(nc, in_maps, core_ids=list(range(NCORE)),
                               trace=PROFILE)
    global LAST_EXEC_NS, LAST_TRACE
    LAST_EXEC_NS = res.exec_time_ns
    LAST_TRACE = res.instructions_and_trace[1] if res.instructions_and_trace else None
    out_slots = np.concatenate([res.results[c]["out"] for c in range(NCORE)], axis=0)
    return out_slots[slot_of_node].astype(np.float32)


def kernel(fluidFeatures, edge_i, edge_j, edge_attr, conv_ws, fc_ws, fc_bs):
    pre = _preprocess(edge_i, edge_j, edge_attr)
    return _build_and_run(pre, fluidFeatures, conv_ws, fc_ws, fc_bs)



# revision 23
# speedup vs baseline: 2.3072x; 1.0974x over previous
"""BasisNetwork (continuous-conv GNN) on 8 Trainium2 NeuronCores.

Per layer (nodes dest-sharded across cores, all compute in bf16/psum-f32):
  out[i] = sum_{e->i} (phi[e] (x) x[j_e]) @ Wflat  +  x[i] @ fc_w + fc_b
The per-edge basis outer product is never materialized.  Edges are host-packed
into groups of <=8 dest nodes / <=128 edge slots; a static block-diagonal
"Sphi" matrix (Sphi[k, slot*16+b] = phi[e_k, b] * [dest(e_k)==slot]) is
streamed from DRAM.  One matmul per group with the raw gathered x_j tile as
the stationary operand:
  M[c, slot*16+b] = sum_k xj[k, c] * Sphi[k, slot*16+b]
then a cheap node-side GEMM per 128 slots:
  out^T[f, slot] = sum_b conv_w[l][b].T @ M[:, slot, b]  (+ fc + bias)
with residual / relu epilogue in slot space.  Activations are all-gathered
(bf16) between layers; x_j rows are fetched with GPSIMD dma_gather from a
256B-padded table.
"""

import numpy as np

# ---------------- problem constants (hardcoded per contract) ----------------
N_NODES = 20000
F = 32               # feature width, all layers
NB = 4
B = NB * NB          # 16 basis functions
N_LAYERS = 4
OUT_SCALE = 1.0 / 128.0
NCORE = 8
NSH = N_NODES // NCORE       # 2500 dest nodes per core
GS = 8                       # node slots per group
GE = 128                     # edge slots per group
GCOL = GS * B                # 128 columns per group (slot*16+b)
EP = 128                     # padded row length (256B in bf16) for dma_gather
GCHUNK = 8                   # groups per Sphi DMA / gather chunk (1024 idxs, HW limit)
NQ = 8                       # dma_gather chunks per layer
DEBUG_STAGE = 0              # 0=full, 1=stop after scatter-GEMM, 2=no epilogue
PROFILE = False              # time warm executes
REPS = 1                     # in-NEFF repetitions of the whole network (For_i)
N_TIMED_EXECS = 8
LAST_EXEC_NS = None
LAST_TRACE = None
LAST_EXEC_TIMES = None


def _hat(x, n):
    c = np.linspace(-1.0, 1.0, n, dtype=np.float32)
    r = np.abs(x[:, None] - c[None, :]) * ((n - 1) * 0.5)
    return np.maximum(1.0 - r, 0.0).astype(np.float32)


def _edge_basis(edge_attr, edge_i, edge_j):
    d = np.clip(edge_attr.astype(np.float32), -1.0, 1.0)
    phi = (_hat(d[:, 0], NB)[:, :, None] * _hat(d[:, 1], NB)[:, None, :]).reshape(-1, B)
    phi *= (edge_i != edge_j).astype(np.float32)[:, None]
    return phi  # [E, B]


def _pack_groups_ffd(nodes, degs):
    """First-fit-decreasing bin packing: nodes into groups with <=GS slots and
    <=GE total edges.  Returns list of groups (each a list of node ids)."""
    order = sorted(range(len(nodes)), key=lambda i: -degs[i])
    bins = []          # (slots_used, edges_used, [nodes])
    for i in order:
        d = degs[i]
        placed = False
        for b in bins:
            if b[0] < GS and b[1] + d <= GE:
                b[0] += 1
                b[1] += d
                b[2].append(nodes[i])
                placed = True
                break
        if not placed:
            bins.append([1, d, [nodes[i]]])
    return [b[2] for b in bins]


def _preprocess(edge_i, edge_j, edge_attr):
    ei = np.asarray(edge_i).astype(np.int64)
    ej = np.asarray(edge_j).astype(np.int64)
    phi = _edge_basis(np.asarray(edge_attr), ei, ej)

    deg = np.bincount(ei, minlength=N_NODES)
    packs = []
    ng_max = 0
    for c in range(NCORE):
        nodes = list(range(c * NSH, (c + 1) * NSH))
        degs = [int(deg[n]) for n in nodes]
        groups = _pack_groups_ffd(nodes, degs)
        packs.append(groups)
        ng_max = max(ng_max, len(groups))

    NG = -(-ng_max // 16) * 16      # multiple of 16 (sphi chunks, gather chunks, batches)
    NSLOT = NG * GS
    assert NCORE * NSLOT < 32768    # int16 gather indices

    # node-GEMM batches (<=512 slots each) and the 2-way allgather split point
    widths = []
    rem = NSLOT
    while rem > 512:
        widths.append(512)
        rem -= 512
    widths.append(rem)
    # one collective part per node batch: each allgather is issued as soon as
    # its batch's activations land, hiding all but the last small collective
    parts = []
    off = 0
    for w in widths:
        parts.append((off, off + w))
        off += w

    # table (gather-source) row numbering: part-major, then core, then slot
    cat2tab = np.zeros(NCORE * NSLOT, dtype=np.int64)
    base = 0
    for (s0, s1) in parts:
        ln = s1 - s0
        for c in range(NCORE):
            cat = c * NSLOT + s0
            cat2tab[cat:cat + ln] = base + c * ln + np.arange(ln)
        base += NCORE * ln

    slot_of_node = np.zeros(N_NODES, dtype=np.int64)
    for c in range(NCORE):
        for gi, nodes in enumerate(packs[c]):
            for s, n in enumerate(nodes):
                slot_of_node[n] = c * NSLOT + gi * GS + s

    # per-core tables
    sphi = np.zeros((NCORE, NG, GE, GCOL), dtype=np.float32)
    gsrc = np.zeros((NCORE, GE, NG), dtype=np.int16)   # [edge slot p, group] -> src row
    order_all = np.argsort(ei, kind="stable")
    starts = np.zeros(N_NODES + 1, dtype=np.int64)
    np.cumsum(deg, out=starts[1:])
    for c in range(NCORE):
        for gi, nodes in enumerate(packs[c]):
            k = 0
            for s, n in enumerate(nodes):
                for e in order_all[starts[n]:starts[n + 1]]:
                    sphi[c, gi, k, s * B:(s + 1) * B] = phi[e]
                    gsrc[c, k, gi] = slot_of_node[ej[e]]
                    k += 1
            assert k <= GE

    # wrapped gather-index layout: stream k = gi*128 + p -> [k%16, k//16], x8 cores
    # (indices in TABLE row order; gsrc itself stays in cat order for xq0)
    idx_w = np.zeros((NCORE, 16, NG * GE // 16), dtype=np.int16)
    for c in range(NCORE):
        stream = cat2tab[gsrc[c].T.reshape(-1).astype(np.int64)]  # k = gi*128 + p
        idx_w[c, :, :] = stream.astype(np.int16).reshape(-1, 16).T
    idx_w = np.tile(idx_w, (1, 8, 1))           # [NCORE, 128, NG*8]

    return NG, NSLOT, slot_of_node, sphi, idx_w, gsrc, widths, parts


def _build_and_run(pre, fluidFeatures, conv_ws, fc_ws, fc_bs):
    import ml_dtypes
    import concourse.bass as bass
    import concourse.bacc as bacc
    import concourse.mybir as mybir
    import concourse.tile as tile
    from concourse.bass_utils import run_bass_kernel_spmd
    from concourse.masks import make_identity
    from concourse import library_config

    NG, NSLOT, slot_of_node, sphi, idx_w, gsrc, widths, parts = pre
    NCHUNK = NG // GCHUNK
    NROWS = NCORE * NSLOT
    # per-batch slot offsets
    boffs = [0]
    for w in widths:
        boffs.append(boffs[-1] + w)
    # table row ranges per collective part
    tab_ranges = []
    base = 0
    for (s0, s1) in parts:
        ln = s1 - s0
        tab_ranges.append((base, base + NCORE * ln))
        base += NCORE * ln

    bf16 = mybir.dt.bfloat16
    f32 = mybir.dt.float32
    i16 = mybir.dt.int16

    x0 = np.asarray(fluidFeatures, dtype=np.float32)
    x0_slots = np.zeros((NROWS, F), dtype=np.float32)
    x0_slots[slot_of_node] = x0

    conv_w = np.stack([np.asarray(w, dtype=np.float32) for w in conv_ws])  # [L,B,F,F]
    fc_w = np.stack([np.asarray(w, dtype=np.float32) for w in fc_ws])      # [L,F,F]
    fc_b = np.stack([np.asarray(b, dtype=np.float32) for b in fc_bs])      # [L,F]

    # ------------------------------ build graph ------------------------------
    nc = bacc.Bacc("TRN2", target_bir_lowering=False, debug=False, num_devices=NCORE,
                   num_swdge_queues=4)

    d_xq0 = nc.dram_tensor("xq0", [GE, NG * F], bf16, kind="ExternalInput")
    d_x0T = nc.dram_tensor("x0T", [F, NSLOT], bf16, kind="ExternalInput")
    d_sphi = nc.dram_tensor("sphi", [GE, NG * GCOL], bf16, kind="ExternalInput")
    d_gidx = nc.dram_tensor("gidx", [128, NG * GE // 16], i16, kind="ExternalInput")
    d_convw = nc.dram_tensor("convw", [F, N_LAYERS * B * F], bf16, kind="ExternalInput")
    d_fcw = nc.dram_tensor("fcw", [F, N_LAYERS * F], bf16, kind="ExternalInput")
    d_fcb = nc.dram_tensor("fcb", [F, N_LAYERS], f32, kind="ExternalInput")
    d_out = nc.dram_tensor("out", [NSLOT, F], f32, kind="ExternalOutput")
    d_ash = [nc.dram_tensor(f"a_shard{h}", [s1 - s0, F], bf16, kind="Internal")
             for h, (s0, s1) in enumerate(parts)]
    d_afull = [nc.dram_tensor(f"a_full{h}", [NCORE * (s1 - s0), F], bf16,
                              kind="Internal", addr_space="Shared")
               for h, (s0, s1) in enumerate(parts)]
    d_apad = nc.dram_tensor("a_pad", [NROWS, EP], bf16, kind="Internal")
    d_wu_in = nc.dram_tensor("wu_in", [128, 16], bf16, kind="Internal")
    d_wu_out = nc.dram_tensor("wu_out", [NCORE * 128, 16], bf16, kind="Internal",
                              addr_space="Shared")

    with tile.TileContext(nc) as tc:
        with (
            tc.tile_pool(name="persist", bufs=1) as pp,
            tc.tile_pool(name="sphi", bufs=3) as sp,
            tc.tile_pool(name="xj", bufs=6) as xp,
            tc.tile_pool(name="work", bufs=3) as wp,
            tc.tile_pool(name="psc", bufs=3, space="PSUM") as psc,
            tc.tile_pool(name="pout", bufs=2, space="PSUM") as pout,
            tc.tile_pool(name="ptr", bufs=1, space="PSUM") as ptr,
        ):
            nc.gpsimd.load_library(library_config.mlp)

            # tiny warmup collective: absorbs first-collective link setup cost
            # while layer-0 compute runs
            nc.gpsimd.collective_compute(
                "AllGather", mybir.AluOpType.bypass,
                replica_groups=[list(range(NCORE))],
                ins=[d_wu_in[:]], outs=[d_wu_out[:]])

            gidx_sb = pp.tile([128, NG * GE // 16], i16)
            nc.sync.dma_start(out=gidx_sb[:], in_=d_gidx[:])
            convw_sb = pp.tile([F, N_LAYERS * B * F], bf16)
            nc.sync.dma_start(out=convw_sb[:], in_=d_convw[:])
            fcw_sb = pp.tile([F, N_LAYERS * F], bf16)
            nc.sync.dma_start(out=fcw_sb[:], in_=d_fcw[:])
            fcb_sb = pp.tile([F, N_LAYERS], f32)
            nc.sync.dma_start(out=fcb_sb[:], in_=d_fcb[:])
            id_bf = pp.tile([F, F], bf16)
            make_identity(nc, id_bf[:])
            id_f32 = pp.tile([F, F], f32)
            make_identity(nc, id_f32[:])
            m_all = pp.tile([F, NSLOT * B], bf16)     # [c, slot*16+b]
            ansT = pp.tile([F, NSLOT], f32)
            aT = pp.tile([F, NSLOT], bf16)
            nc.sync.dma_start(out=aT[:], in_=d_x0T[:])
            xq0_sb = pp.tile([GE, NG * F], bf16)      # layer-0 x_j, host pre-gathered
            nc.sync.dma_start(out=xq0_sb[:], in_=d_xq0[:])

            for l in range(N_LAYERS):
                if l > 0:
                    for h, (t0, t1) in enumerate(tab_ranges):
                        nc.gpsimd.collective_compute(
                            "AllGather", mybir.AluOpType.bypass,
                            replica_groups=[list(range(NCORE))],
                            ins=[d_ash[h][:]], outs=[d_afull[h][:]])
                        nc.sync.dma_start(out=d_apad[t0:t1, 0:F],
                                          in_=d_afull[h][:])

                # ---------- gather + scatter-GEMM, chunked by GCHUNK groups ----------
                NIC = GCHUNK * GE             # gather idxs per chunk
                for ch in range(NCHUNK):
                    if l == 0:
                        xq = xq0_sb
                        xofs, xstride = ch * GCHUNK * F, F
                    else:
                        xq = xp.tile([GE, GCHUNK * EP], bf16, tag="xj")
                        xofs, xstride = 0, EP
                        nc.gpsimd.dma_gather(
                            out_ap=xq[:].rearrange("p (g e) -> p g e", e=EP),
                            in_ap=d_apad[:],
                            idxs_ap=gidx_sb[:, ch * NIC // 16:(ch + 1) * NIC // 16],
                            num_idxs=NIC, num_idxs_reg=NIC, elem_size=EP,
                            queue_num=ch % 4)
                    st = sp.tile([GE, GCHUNK * GCOL], bf16, tag="st")
                    nc.sync.dma_start(
                        out=st[:],
                        in_=d_sphi[:, ch * GCHUNK * GCOL:(ch + 1) * GCHUNK * GCOL])
                    for g4 in range(GCHUNK // 4):
                        ps = psc.tile([F, 4 * GCOL], f32, tag="psc")
                        for j in range(4):
                            gl = g4 * 4 + j
                            nc.tensor.matmul(
                                out=ps[:, j * GCOL:(j + 1) * GCOL],
                                lhsT=xq[:, xofs + gl * xstride:xofs + gl * xstride + F],
                                rhs=st[:, gl * GCOL:(gl + 1) * GCOL],
                                start=True, stop=True)
                        gi0 = ch * GCHUNK + g4 * 4
                        if g4 % 2 == 0:
                            nc.scalar.copy(
                                out=m_all[:, gi0 * GCOL:(gi0 + 4) * GCOL], in_=ps[:])
                        else:
                            nc.vector.tensor_copy(
                                out=m_all[:, gi0 * GCOL:(gi0 + 4) * GCOL], in_=ps[:])

                # ---------- node-side GEMM + epilogue per <=512 slots ----------
                if DEBUG_STAGE == 1:
                    continue
                m_view = m_all[:].rearrange("c (s b) -> c b s", b=B)
                for bi, w in enumerate(widths):
                    sl = slice(boffs[bi], boffs[bi] + w)
                    nk = w // 128
                    po = pout.tile([F, w], f32, tag="pout")
                    for b in range(B):
                        wofs = (l * B + b) * F
                        nc.tensor.matmul(
                            out=po[:], lhsT=convw_sb[:, wofs:wofs + F],
                            rhs=m_view[:, b, sl], start=(b == 0), stop=False)
                    nc.tensor.matmul(
                        out=po[:], lhsT=fcw_sb[:, l * F:(l + 1) * F],
                        rhs=aT[:, sl], start=False, stop=True)

                    if DEBUG_STAGE == 2:
                        continue
                    if l == 0:
                        nc.scalar.activation(
                            out=ansT[:, sl], in_=po[:],
                            func=mybir.ActivationFunctionType.Identity,
                            bias=fcb_sb[:, l:l + 1])
                    else:
                        tmp = wp.tile([F, w], f32, tag="tmp")
                        nc.scalar.activation(
                            out=tmp[:], in_=po[:],
                            func=mybir.ActivationFunctionType.Identity,
                            bias=fcb_sb[:, l:l + 1])
                        nc.vector.tensor_add(
                            out=ansT[:, sl], in0=ansT[:, sl], in1=tmp[:])

                    if l < N_LAYERS - 1:
                        nc.scalar.activation(
                            out=aT[:, sl], in_=ansT[:, sl],
                            func=mybir.ActivationFunctionType.Relu)
                        aout = wp.tile([128, nk * F], bf16, tag="aout")
                        for k in range(nk):
                            s2 = slice(boffs[bi] + k * 128, boffs[bi] + (k + 1) * 128)
                            pt = ptr.tile([128, F], bf16, tag="ptrb")
                            nc.tensor.transpose(out=pt[:], in_=aT[:, s2],
                                                identity=id_bf[:])
                            nc.vector.tensor_copy(
                                out=aout[:, k * F:(k + 1) * F], in_=pt[:])
                        nc.sync.dma_start(
                            out=d_ash[bi][:].rearrange(
                                "(k p) f -> p k f", p=128),
                            in_=aout[:].rearrange("p (k f) -> p k f", f=F))
                    else:
                        oT = wp.tile([F, w], f32, tag="oT")
                        nc.scalar.activation(
                            out=oT[:], in_=ansT[:, sl],
                            func=mybir.ActivationFunctionType.Copy,
                            scale=OUT_SCALE)
                        oout = wp.tile([128, nk * F], f32, tag="oout")
                        for k in range(nk):
                            pt = ptr.tile([128, F], f32, tag="ptr")
                            nc.tensor.transpose(out=pt[:], in_=oT[:, k * 128:(k + 1) * 128],
                                                identity=id_f32[:])
                            nc.vector.tensor_copy(
                                out=oout[:, k * F:(k + 1) * F], in_=pt[:])
                        nc.sync.dma_start(
                            out=d_out[sl, :].rearrange("(k p) f -> p k f", p=128),
                            in_=oout[:].rearrange("p (k f) -> p k f", f=F))

    nc.compile()

    # ------------------------------ run ------------------------------
    convw_c = np.ascontiguousarray(
        conv_w.transpose(2, 0, 1, 3).reshape(F, N_LAYERS * B * F)
    ).astype(ml_dtypes.bfloat16)
    fcw_c = np.ascontiguousarray(
        fc_w.transpose(1, 0, 2).reshape(F, N_LAYERS * F)).astype(ml_dtypes.bfloat16)
    fcb_T = np.ascontiguousarray(fc_b.T)
    sphi_bf = sphi.astype(ml_dtypes.bfloat16)
    x0_slots_bf = x0_slots.astype(ml_dtypes.bfloat16)

    in_maps = []
    for c in range(NCORE):
        x0T_c = np.ascontiguousarray(
            x0_slots[c * NSLOT:(c + 1) * NSLOT].T).astype(ml_dtypes.bfloat16)
        sphi_c = np.ascontiguousarray(
            sphi_bf[c].transpose(1, 0, 2).reshape(GE, NG * GCOL))
        # layer-0 x_j gathered on host: xq0[p, gi*F:(gi+1)*F] = x0[gsrc[c][p, gi]]
        xq0_c = np.ascontiguousarray(
            x0_slots_bf[gsrc[c].T.astype(np.int64)].transpose(1, 0, 2)
            .reshape(GE, NG * F))
        in_maps.append({
            "xq0": xq0_c,
            "x0T": x0T_c,
            "sphi": sphi_c,
            "gidx": np.ascontiguousarray(idx_w[c]),
            "convw": convw_c,
            "fcw": fcw_c,
            "fcb": fcb_T,
        })

    res = run_bass_kernel_spmd(nc, in_maps, core_ids=list(range(NCORE)),
                               trace=PROFILE)
    global LAST_EXEC_NS, LAST_TRACE
    LAST_EXEC_NS = res.exec_time_ns
    LAST_TRACE = res.instructions_and_trace[1] if res.instructions_and_trace else None
    out_slots = np.concatenate([res.results[c]["out"] for c in range(NCORE)], axis=0)
    return out_slots[slot_of_node].astype(np.float32)


def kernel(fluidFeatures, edge_i, edge_j, edge_attr, conv_ws, fc_ws, fc_bs):
    pre = _preprocess(edge_i, edge_j, edge_attr)
    return _build_and_run(pre, fluidFeatures, conv_ws, fc_ws, fc_bs)



# revision 24
# speedup vs baseline: 2.3674x; 1.0261x over previous
"""BasisNetwork (continuous-conv GNN) on 8 Trainium2 NeuronCores.

Per layer (nodes dest-sharded across cores, all compute in bf16/psum-f32):
  out[i] = sum_{e->i} (phi[e] (x) x[j_e]) @ Wflat  +  x[i] @ fc_w + fc_b
The per-edge basis outer product is never materialized.  Edges are host-packed
into groups of <=8 dest nodes / <=128 edge slots; a static block-diagonal
"Sphi" matrix (Sphi[k, slot*16+b] = phi[e_k, b] * [dest(e_k)==slot]) is
streamed from DRAM.  One matmul per group with the raw gathered x_j tile as
the stationary operand:
  M[c, slot*16+b] = sum_k xj[k, c] * Sphi[k, slot*16+b]
then a cheap node-side GEMM per 128 slots:
  out^T[f, slot] = sum_b conv_w[l][b].T @ M[:, slot, b]  (+ fc + bias)
with residual / relu epilogue in slot space.  Activations are all-gathered
(bf16) between layers; x_j rows are fetched with GPSIMD dma_gather from a
256B-padded table.
"""

import numpy as np

# ---------------- problem constants (hardcoded per contract) ----------------
N_NODES = 20000
F = 32               # feature width, all layers
NB = 4
B = NB * NB          # 16 basis functions
N_LAYERS = 4
OUT_SCALE = 1.0 / 128.0
NCORE = 8
NSH = N_NODES // NCORE       # 2500 dest nodes per core
GS = 8                       # node slots per group
GE = 128                     # edge slots per group
GCOL = GS * B                # 128 columns per group (slot*16+b)
EP = 128                     # padded row length (256B in bf16) for dma_gather
GCHUNK = 8                   # groups per Sphi DMA / gather chunk (1024 idxs, HW limit)
NQ = 8                       # dma_gather chunks per layer
DEBUG_STAGE = 0              # 0=full, 1=stop after scatter-GEMM, 2=no epilogue
PROFILE = False              # time warm executes
REPS = 1                     # in-NEFF repetitions of the whole network (For_i)
N_TIMED_EXECS = 8
LAST_EXEC_NS = None
LAST_TRACE = None
LAST_EXEC_TIMES = None


def _hat(x, n):
    c = np.linspace(-1.0, 1.0, n, dtype=np.float32)
    r = np.abs(x[:, None] - c[None, :]) * ((n - 1) * 0.5)
    return np.maximum(1.0 - r, 0.0).astype(np.float32)


def _edge_basis(edge_attr, edge_i, edge_j):
    d = np.clip(edge_attr.astype(np.float32), -1.0, 1.0)
    phi = (_hat(d[:, 0], NB)[:, :, None] * _hat(d[:, 1], NB)[:, None, :]).reshape(-1, B)
    phi *= (edge_i != edge_j).astype(np.float32)[:, None]
    return phi  # [E, B]


def _pack_groups_ffd(nodes, degs):
    """First-fit-decreasing bin packing: nodes into groups with <=GS slots and
    <=GE total edges.  Returns list of groups (each a list of node ids)."""
    order = sorted(range(len(nodes)), key=lambda i: -degs[i])
    bins = []          # (slots_used, edges_used, [nodes])
    for i in order:
        d = degs[i]
        placed = False
        for b in bins:
            if b[0] < GS and b[1] + d <= GE:
                b[0] += 1
                b[1] += d
                b[2].append(nodes[i])
                placed = True
                break
        if not placed:
            bins.append([1, d, [nodes[i]]])
    return [b[2] for b in bins]


def _preprocess(edge_i, edge_j, edge_attr):
    ei = np.asarray(edge_i).astype(np.int64)
    ej = np.asarray(edge_j).astype(np.int64)
    phi = _edge_basis(np.asarray(edge_attr), ei, ej)

    deg = np.bincount(ei, minlength=N_NODES)
    packs = []
    ng_max = 0
    for c in range(NCORE):
        nodes = list(range(c * NSH, (c + 1) * NSH))
        degs = [int(deg[n]) for n in nodes]
        groups = _pack_groups_ffd(nodes, degs)
        packs.append(groups)
        ng_max = max(ng_max, len(groups))

    NG = -(-ng_max // 16) * 16      # multiple of 16 (sphi chunks, gather chunks, batches)
    NSLOT = NG * GS
    assert NCORE * NSLOT < 32768    # int16 gather indices

    # node-GEMM batches (<=512 slots each) and the 2-way allgather split point
    widths = []
    rem = NSLOT
    while rem > 512:
        widths.append(512)
        rem -= 512
    widths.append(rem)
    # one collective part per node batch: each allgather is issued as soon as
    # its batch's activations land, hiding all but the last small collective
    parts = []
    off = 0
    for w in widths:
        parts.append((off, off + w))
        off += w

    # table (gather-source) row numbering: part-major, then core, then slot
    cat2tab = np.zeros(NCORE * NSLOT, dtype=np.int64)
    base = 0
    for (s0, s1) in parts:
        ln = s1 - s0
        for c in range(NCORE):
            cat = c * NSLOT + s0
            cat2tab[cat:cat + ln] = base + c * ln + np.arange(ln)
        base += NCORE * ln

    slot_of_node = np.zeros(N_NODES, dtype=np.int64)
    for c in range(NCORE):
        for gi, nodes in enumerate(packs[c]):
            for s, n in enumerate(nodes):
                slot_of_node[n] = c * NSLOT + gi * GS + s

    # per-core tables
    sphi = np.zeros((NCORE, NG, GE, GCOL), dtype=np.float32)
    gsrc = np.zeros((NCORE, GE, NG), dtype=np.int16)   # [edge slot p, group] -> src row
    order_all = np.argsort(ei, kind="stable")
    starts = np.zeros(N_NODES + 1, dtype=np.int64)
    np.cumsum(deg, out=starts[1:])
    for c in range(NCORE):
        for gi, nodes in enumerate(packs[c]):
            k = 0
            for s, n in enumerate(nodes):
                for e in order_all[starts[n]:starts[n + 1]]:
                    sphi[c, gi, k, s * B:(s + 1) * B] = phi[e]
                    gsrc[c, k, gi] = slot_of_node[ej[e]]
                    k += 1
            assert k <= GE

    # wrapped gather-index layout: stream k = gi*128 + p -> [k%16, k//16], x8 cores
    # (indices in TABLE row order; gsrc itself stays in cat order for xq0)
    idx_w = np.zeros((NCORE, 16, NG * GE // 16), dtype=np.int16)
    for c in range(NCORE):
        stream = cat2tab[gsrc[c].T.reshape(-1).astype(np.int64)]  # k = gi*128 + p
        idx_w[c, :, :] = stream.astype(np.int16).reshape(-1, 16).T
    idx_w = np.tile(idx_w, (1, 8, 1))           # [NCORE, 128, NG*8]

    return NG, NSLOT, slot_of_node, sphi, idx_w, gsrc, widths, parts


def _build_and_run(pre, fluidFeatures, conv_ws, fc_ws, fc_bs):
    import ml_dtypes
    import concourse.bass as bass
    import concourse.bacc as bacc
    import concourse.mybir as mybir
    import concourse.tile as tile
    from concourse.bass_utils import run_bass_kernel_spmd
    from concourse.masks import make_identity
    from concourse import library_config

    NG, NSLOT, slot_of_node, sphi, idx_w, gsrc, widths, parts = pre
    NCHUNK = NG // GCHUNK
    NROWS = NCORE * NSLOT
    # per-batch slot offsets
    boffs = [0]
    for w in widths:
        boffs.append(boffs[-1] + w)
    # table row ranges per collective part
    tab_ranges = []
    base = 0
    for (s0, s1) in parts:
        ln = s1 - s0
        tab_ranges.append((base, base + NCORE * ln))
        base += NCORE * ln

    bf16 = mybir.dt.bfloat16
    f32 = mybir.dt.float32
    i16 = mybir.dt.int16

    x0 = np.asarray(fluidFeatures, dtype=np.float32)
    x0_slots = np.zeros((NROWS, F), dtype=np.float32)
    x0_slots[slot_of_node] = x0

    conv_w = np.stack([np.asarray(w, dtype=np.float32) for w in conv_ws])  # [L,B,F,F]
    fc_w = np.stack([np.asarray(w, dtype=np.float32) for w in fc_ws])      # [L,F,F]
    fc_b = np.stack([np.asarray(b, dtype=np.float32) for b in fc_bs])      # [L,F]

    # ------------------------------ build graph ------------------------------
    nc = bacc.Bacc("TRN2", target_bir_lowering=False, debug=False, num_devices=NCORE,
                   num_swdge_queues=4)

    d_xq0 = nc.dram_tensor("xq0", [GE, NG * F], bf16, kind="ExternalInput")
    d_x0T = nc.dram_tensor("x0T", [F, NSLOT], bf16, kind="ExternalInput")
    d_sphi = nc.dram_tensor("sphi", [GE, NG * GCOL], bf16, kind="ExternalInput")
    d_gidx = nc.dram_tensor("gidx", [128, NG * GE // 16], i16, kind="ExternalInput")
    d_convw = nc.dram_tensor("convw", [F, N_LAYERS * B * F], bf16, kind="ExternalInput")
    d_fcw = nc.dram_tensor("fcw", [F, N_LAYERS * F], bf16, kind="ExternalInput")
    d_fcb = nc.dram_tensor("fcb", [F, N_LAYERS], f32, kind="ExternalInput")
    d_out = nc.dram_tensor("out", [NSLOT, F], f32, kind="ExternalOutput")
    d_ash = [nc.dram_tensor(f"a_shard{h}", [s1 - s0, F], bf16, kind="Internal")
             for h, (s0, s1) in enumerate(parts)]
    d_afull = [nc.dram_tensor(f"a_full{h}", [NCORE * (s1 - s0), F], bf16,
                              kind="Internal", addr_space="Shared")
               for h, (s0, s1) in enumerate(parts)]
    d_apad = nc.dram_tensor("a_pad", [NROWS, EP], bf16, kind="Internal")
    d_wu_in = nc.dram_tensor("wu_in", [128, 16], bf16, kind="Internal")
    d_wu_out = nc.dram_tensor("wu_out", [NCORE * 128, 16], bf16, kind="Internal",
                              addr_space="Shared")

    with tile.TileContext(nc) as tc:
        with (
            tc.tile_pool(name="persist", bufs=1) as pp,
            tc.tile_pool(name="sphi", bufs=3) as sp,
            tc.tile_pool(name="xj", bufs=6) as xp,
            tc.tile_pool(name="work", bufs=3) as wp,
            tc.tile_pool(name="psc", bufs=3, space="PSUM") as psc,
            tc.tile_pool(name="pout", bufs=2, space="PSUM") as pout,
            tc.tile_pool(name="ptr", bufs=1, space="PSUM") as ptr,
        ):
            nc.gpsimd.load_library(library_config.mlp)

            # tiny warmup collective: absorbs first-collective link setup cost
            # while layer-0 compute runs
            nc.gpsimd.collective_compute(
                "AllGather", mybir.AluOpType.bypass,
                replica_groups=[list(range(NCORE))],
                ins=[d_wu_in[:]], outs=[d_wu_out[:]])

            gidx_sb = pp.tile([128, NG * GE // 16], i16)
            nc.sync.dma_start(out=gidx_sb[:], in_=d_gidx[:])
            convw_sb = pp.tile([F, N_LAYERS * B * F], bf16)
            nc.sync.dma_start(out=convw_sb[:], in_=d_convw[:])
            fcw_sb = pp.tile([F, N_LAYERS * F], bf16)
            nc.sync.dma_start(out=fcw_sb[:], in_=d_fcw[:])
            fcb_sb = pp.tile([F, N_LAYERS], f32)
            nc.sync.dma_start(out=fcb_sb[:], in_=d_fcb[:])
            id_bf = pp.tile([F, F], bf16)
            make_identity(nc, id_bf[:])
            id_f32 = pp.tile([F, F], f32)
            make_identity(nc, id_f32[:])
            m_all = pp.tile([F, NSLOT * B], bf16)     # [c, slot*16+b]
            ansT = pp.tile([F, NSLOT], f32)
            aT = pp.tile([F, NSLOT], bf16)
            nc.sync.dma_start(out=aT[:], in_=d_x0T[:])
            xq0_sb = pp.tile([GE, NG * F], bf16)      # layer-0 x_j, host pre-gathered
            nc.sync.dma_start(out=xq0_sb[:], in_=d_xq0[:])

            for l in range(N_LAYERS):
                if l > 0:
                    for h, (t0, t1) in enumerate(tab_ranges):
                        nc.gpsimd.collective_compute(
                            "AllGather", mybir.AluOpType.bypass,
                            replica_groups=[list(range(NCORE))],
                            ins=[d_ash[h][:]], outs=[d_afull[h][:]])
                        nc.sync.dma_start(out=d_apad[t0:t1, 0:F],
                                          in_=d_afull[h][:])

                # ---------- gather + scatter-GEMM, chunked by GCHUNK groups ----------
                NIC = GCHUNK * GE             # gather idxs per chunk
                for ch in range(NCHUNK):
                    if l == 0:
                        xq = xq0_sb
                        xofs, xstride = ch * GCHUNK * F, F
                    else:
                        xq = xp.tile([GE, GCHUNK * EP], bf16, tag="xj")
                        xofs, xstride = 0, EP
                        nc.gpsimd.dma_gather(
                            out_ap=xq[:].rearrange("p (g e) -> p g e", e=EP),
                            in_ap=d_apad[:],
                            idxs_ap=gidx_sb[:, ch * NIC // 16:(ch + 1) * NIC // 16],
                            num_idxs=NIC, num_idxs_reg=NIC, elem_size=EP,
                            queue_num=ch % 4)
                    st = sp.tile([GE, GCHUNK * GCOL], bf16, tag="st")
                    nc.sync.dma_start(
                        out=st[:],
                        in_=d_sphi[:, ch * GCHUNK * GCOL:(ch + 1) * GCHUNK * GCOL])
                    for g4 in range(GCHUNK // 4):
                        # 4 groups packed into the 4 PE column-groups: the
                        # matmuls run concurrently (separate XBUS streams) and
                        # land in 4 partition strips of one PSUM tile
                        ps = psc.tile([128, GCOL], f32, tag="psc")
                        for j in range(4):
                            gl = g4 * 4 + j
                            nc.tensor.matmul(
                                out=ps[32 * j:32 * j + 32, :],
                                lhsT=xq[:, xofs + gl * xstride:xofs + gl * xstride + F],
                                rhs=st[:, gl * GCOL:(gl + 1) * GCOL],
                                start=True, stop=True, tile_position=(0, 32 * j))
                        gi0 = ch * GCHUNK + g4 * 4
                        for j in range(4):
                            src = ps[32 * j:32 * j + 32, :]
                            dst = m_all[:, (gi0 + j) * GCOL:(gi0 + j + 1) * GCOL]
                            if j % 2 == 0:
                                nc.scalar.copy(out=dst, in_=src)
                            else:
                                nc.vector.tensor_copy(out=dst, in_=src)

                # ---------- node-side GEMM + epilogue per <=512 slots ----------
                if DEBUG_STAGE == 1:
                    continue
                m_view = m_all[:].rearrange("c (s b) -> c b s", b=B)
                for bi, w in enumerate(widths):
                    sl = slice(boffs[bi], boffs[bi] + w)
                    nk = w // 128
                    po = pout.tile([F, w], f32, tag="pout")
                    for b in range(B):
                        wofs = (l * B + b) * F
                        nc.tensor.matmul(
                            out=po[:], lhsT=convw_sb[:, wofs:wofs + F],
                            rhs=m_view[:, b, sl], start=(b == 0), stop=False)
                    nc.tensor.matmul(
                        out=po[:], lhsT=fcw_sb[:, l * F:(l + 1) * F],
                        rhs=aT[:, sl], start=False, stop=True)

                    if DEBUG_STAGE == 2:
                        continue
                    if l == 0:
                        nc.scalar.activation(
                            out=ansT[:, sl], in_=po[:],
                            func=mybir.ActivationFunctionType.Identity,
                            bias=fcb_sb[:, l:l + 1])
                    else:
                        tmp = wp.tile([F, w], f32, tag="tmp")
                        nc.scalar.activation(
                            out=tmp[:], in_=po[:],
                            func=mybir.ActivationFunctionType.Identity,
                            bias=fcb_sb[:, l:l + 1])
                        nc.vector.tensor_add(
                            out=ansT[:, sl], in0=ansT[:, sl], in1=tmp[:])

                    if l < N_LAYERS - 1:
                        nc.scalar.activation(
                            out=aT[:, sl], in_=ansT[:, sl],
                            func=mybir.ActivationFunctionType.Relu)
                        aout = wp.tile([128, nk * F], bf16, tag="aout")
                        for k in range(nk):
                            s2 = slice(boffs[bi] + k * 128, boffs[bi] + (k + 1) * 128)
                            pt = ptr.tile([128, F], bf16, tag="ptrb")
                            nc.tensor.transpose(out=pt[:], in_=aT[:, s2],
                                                identity=id_bf[:])
                            nc.vector.tensor_copy(
                                out=aout[:, k * F:(k + 1) * F], in_=pt[:])
                        nc.sync.dma_start(
                            out=d_ash[bi][:].rearrange(
                                "(k p) f -> p k f", p=128),
                            in_=aout[:].rearrange("p (k f) -> p k f", f=F))
                    else:
                        oT = wp.tile([F, w], f32, tag="oT")
                        nc.scalar.activation(
                            out=oT[:], in_=ansT[:, sl],
                            func=mybir.ActivationFunctionType.Copy,
                            scale=OUT_SCALE)
                        oout = wp.tile([128, nk * F], f32, tag="oout")
                        for k in range(nk):
                            pt = ptr.tile([128, F], f32, tag="ptr")
                            nc.tensor.transpose(out=pt[:], in_=oT[:, k * 128:(k + 1) * 128],
                                                identity=id_f32[:])
                            nc.vector.tensor_copy(
                                out=oout[:, k * F:(k + 1) * F], in_=pt[:])
                        nc.sync.dma_start(
                            out=d_out[sl, :].rearrange("(k p) f -> p k f", p=128),
                            in_=oout[:].rearrange("p (k f) -> p k f", f=F))

    nc.compile()

    # ------------------------------ run ------------------------------
    convw_c = np.ascontiguousarray(
        conv_w.transpose(2, 0, 1, 3).reshape(F, N_LAYERS * B * F)
    ).astype(ml_dtypes.bfloat16)
    fcw_c = np.ascontiguousarray(
        fc_w.transpose(1, 0, 2).reshape(F, N_LAYERS * F)).astype(ml_dtypes.bfloat16)
    fcb_T = np.ascontiguousarray(fc_b.T)
    sphi_bf = sphi.astype(ml_dtypes.bfloat16)
    x0_slots_bf = x0_slots.astype(ml_dtypes.bfloat16)

    in_maps = []
    for c in range(NCORE):
        x0T_c = np.ascontiguousarray(
            x0_slots[c * NSLOT:(c + 1) * NSLOT].T).astype(ml_dtypes.bfloat16)
        sphi_c = np.ascontiguousarray(
            sphi_bf[c].transpose(1, 0, 2).reshape(GE, NG * GCOL))
        # layer-0 x_j gathered on host: xq0[p, gi*F:(gi+1)*F] = x0[gsrc[c][p, gi]]
        xq0_c = np.ascontiguousarray(
            x0_slots_bf[gsrc[c].T.astype(np.int64)].transpose(1, 0, 2)
            .reshape(GE, NG * F))
        in_maps.append({
            "xq0": xq0_c,
            "x0T": x0T_c,
            "sphi": sphi_c,
            "gidx": np.ascontiguousarray(idx_w[c]),
            "convw": convw_c,
            "fcw": fcw_c,
            "fcb": fcb_T,
        })

    res = run_bass_kernel_spmd(nc, in_maps, core_ids=list(range(NCORE)),
                               trace=PROFILE)
    global LAST_EXEC_NS, LAST_TRACE
    LAST_EXEC_NS = res.exec_time_ns
    LAST_TRACE = res.instructions_and_trace[1] if res.instructions_and_trace else None
    out_slots = np.concatenate([res.results[c]["out"] for c in range(NCORE)], axis=0)
    return out_slots[slot_of_node].astype(np.float32)


def kernel(fluidFeatures, edge_i, edge_j, edge_attr, conv_ws, fc_ws, fc_bs):
    pre = _preprocess(edge_i, edge_j, edge_attr)
    return _build_and_run(pre, fluidFeatures, conv_ws, fc_ws, fc_bs)



# revision 34
# speedup vs baseline: 2.5898x; 1.0939x over previous
"""BasisNetwork (continuous-conv GNN) on 8 Trainium2 NeuronCores.

Per layer (nodes dest-sharded across cores, all compute in bf16/psum-f32):
  out[i] = sum_{e->i} (phi[e] (x) x[j_e]) @ Wflat  +  x[i] @ fc_w + fc_b
The per-edge basis outer product is never materialized.  Edges are host-packed
into groups of <=8 dest nodes / <=128 edge slots; a static block-diagonal
"Sphi" matrix (Sphi[k, slot*16+b] = phi[e_k, b] * [dest(e_k)==slot]) is
streamed from DRAM.  One matmul per group with the raw gathered x_j tile as
the stationary operand:
  M[c, slot*16+b] = sum_k xj[k, c] * Sphi[k, slot*16+b]
then a cheap node-side GEMM per 128 slots:
  out^T[f, slot] = sum_b conv_w[l][b].T @ M[:, slot, b]  (+ fc + bias)
with residual / relu epilogue in slot space.  Activations are all-gathered
(bf16) between layers; x_j rows are fetched with GPSIMD dma_gather from a
256B-padded table.
"""

import numpy as np

# ---------------- problem constants (hardcoded per contract) ----------------
N_NODES = 20000
F = 32               # feature width, all layers
NB = 4
B = NB * NB          # 16 basis functions
N_LAYERS = 4
OUT_SCALE = 1.0 / 128.0
NCORE = 8
NSH = N_NODES // NCORE       # 2500 dest nodes per core
GS = 8                       # node slots per group
GE = 128                     # edge slots per group
GCOL = GS * B                # 128 columns per group (slot*16+b)
EP = 128                     # padded row length (256B in bf16) for dma_gather
GCHUNK = 8                   # groups per Sphi DMA / gather chunk (1024 idxs, HW limit)
NQ = 8                       # dma_gather chunks per layer
DEBUG_STAGE = 0              # 0=full, 1=stop after scatter-GEMM, 2=no epilogue
PROFILE = False              # time warm executes
REPS = 1                     # in-NEFF repetitions of the whole network (For_i)
N_TIMED_EXECS = 8
LAST_EXEC_NS = None
LAST_TRACE = None
LAST_EXEC_TIMES = None


def _hat(x, n):
    c = np.linspace(-1.0, 1.0, n, dtype=np.float32)
    r = np.abs(x[:, None] - c[None, :]) * ((n - 1) * 0.5)
    return np.maximum(1.0 - r, 0.0).astype(np.float32)


def _edge_basis(edge_attr, edge_i, edge_j):
    d = np.clip(edge_attr.astype(np.float32), -1.0, 1.0)
    phi = (_hat(d[:, 0], NB)[:, :, None] * _hat(d[:, 1], NB)[:, None, :]).reshape(-1, B)
    phi *= (edge_i != edge_j).astype(np.float32)[:, None]
    return phi  # [E, B]


def _pack_groups_ffd(nodes, degs):
    """First-fit-decreasing bin packing: nodes into groups with <=GS slots and
    <=GE total edges.  Returns list of groups (each a list of node ids)."""
    order = sorted(range(len(nodes)), key=lambda i: -degs[i])
    bins = []          # (slots_used, edges_used, [nodes])
    for i in order:
        d = degs[i]
        placed = False
        for b in bins:
            if b[0] < GS and b[1] + d <= GE:
                b[0] += 1
                b[1] += d
                b[2].append(nodes[i])
                placed = True
                break
        if not placed:
            bins.append([1, d, [nodes[i]]])
    return [b[2] for b in bins]


def _preprocess(edge_i, edge_j, edge_attr):
    ei = np.asarray(edge_i).astype(np.int64)
    ej = np.asarray(edge_j).astype(np.int64)
    phi = _edge_basis(np.asarray(edge_attr), ei, ej)

    deg = np.bincount(ei, minlength=N_NODES)
    packs = []
    ng_max = 0
    for c in range(NCORE):
        nodes = list(range(c * NSH, (c + 1) * NSH))
        degs = [int(deg[n]) for n in nodes]
        groups = _pack_groups_ffd(nodes, degs)
        packs.append(groups)
        ng_max = max(ng_max, len(groups))

    NG = -(-ng_max // 16) * 16      # multiple of 16 (sphi chunks, gather chunks, batches)
    NSLOT = NG * GS
    assert NCORE * NSLOT < 32768    # int16 gather indices

    # node-GEMM batches (<=512 slots each) and the 2-way allgather split point
    widths = []
    rem = NSLOT
    while rem > 512:
        widths.append(512)
        rem -= 512
    widths.append(rem)
    # one collective part per node batch: each allgather is issued as soon as
    # its batch's activations land, hiding all but the last small collective
    parts = []
    off = 0
    for w in widths:
        parts.append((off, off + w))
        off += w

    # table (gather-source) row numbering: batch-major, then core, then
    # strip-stacked order (u*4 + strip) matching the stacked epilogue DMAs.
    # slot = gi*GS + s; strip j = gi % 4; u_local = (gi//4 - first_pack)*GS + s
    slot_ids = np.arange(NSLOT)
    gi_of = slot_ids // GS
    s_of = slot_ids % GS
    j_of = gi_of % 4
    outrow_of_slot = np.zeros(NSLOT, dtype=np.int64)
    cat2tab = np.zeros(NCORE * NSLOT, dtype=np.int64)
    base = 0
    for (s0, s1) in parts:
        ln = s1 - s0
        sel = slot_ids[s0:s1]
        p0 = s0 // (4 * GS)
        u_local = (gi_of[sel] // 4 - p0) * GS + s_of[sel]
        local_row = u_local * 4 + j_of[sel]
        outrow_of_slot[sel] = s0 + local_row
        for c in range(NCORE):
            cat2tab[c * NSLOT + s0:c * NSLOT + s1] = base + c * ln + local_row
        base += NCORE * ln

    slot_of_node = np.zeros(N_NODES, dtype=np.int64)
    for c in range(NCORE):
        for gi, nodes in enumerate(packs[c]):
            for s, n in enumerate(nodes):
                slot_of_node[n] = c * NSLOT + gi * GS + s

    # per-core tables
    sphi = np.zeros((NCORE, NG, GE, GCOL), dtype=np.float32)
    gsrc = np.zeros((NCORE, GE, NG), dtype=np.int16)   # [edge slot p, group] -> src row
    order_all = np.argsort(ei, kind="stable")
    starts = np.zeros(N_NODES + 1, dtype=np.int64)
    np.cumsum(deg, out=starts[1:])
    for c in range(NCORE):
        for gi, nodes in enumerate(packs[c]):
            k = 0
            for s, n in enumerate(nodes):
                for e in order_all[starts[n]:starts[n + 1]]:
                    sphi[c, gi, k, s * B:(s + 1) * B] = phi[e]
                    gsrc[c, k, gi] = slot_of_node[ej[e]]
                    k += 1
            assert k <= GE

    # wrapped gather-index layout: stream k = gi*128 + p -> [k%16, k//16], x8 cores
    # (indices in TABLE row order; gsrc itself stays in cat order for xq0)
    idx_w = np.zeros((NCORE, 16, NG * GE // 16), dtype=np.int16)
    for c in range(NCORE):
        stream = cat2tab[gsrc[c].T.reshape(-1).astype(np.int64)]  # k = gi*128 + p
        idx_w[c, :, :] = stream.astype(np.int16).reshape(-1, 16).T
    idx_w = np.tile(idx_w, (1, 8, 1))           # [NCORE, 128, NG*8]

    return NG, NSLOT, slot_of_node, sphi, idx_w, gsrc, widths, parts, outrow_of_slot


def _build_and_run(pre, fluidFeatures, conv_ws, fc_ws, fc_bs):
    import ml_dtypes
    import concourse.bass as bass
    import concourse.bacc as bacc
    import concourse.mybir as mybir
    import concourse.tile as tile
    from concourse.bass_utils import run_bass_kernel_spmd
    from concourse.masks import make_identity
    from concourse import library_config

    NG, NSLOT, slot_of_node, sphi, idx_w, gsrc, widths, parts, outrow_of_slot = pre
    NCHUNK = NG // GCHUNK
    NROWS = NCORE * NSLOT
    NPACK = NG // 4
    NU = NSLOT // 4              # stacked columns (4 strips in partitions)
    # per-batch slot offsets
    boffs = [0]
    for w in widths:
        boffs.append(boffs[-1] + w)
    # table row ranges per collective part
    tab_ranges = []
    base = 0
    for (s0, s1) in parts:
        ln = s1 - s0
        tab_ranges.append((base, base + NCORE * ln))
        base += NCORE * ln

    bf16 = mybir.dt.bfloat16
    f32 = mybir.dt.float32
    i16 = mybir.dt.int16

    x0 = np.asarray(fluidFeatures, dtype=np.float32)
    x0_slots = np.zeros((NROWS, F), dtype=np.float32)
    x0_slots[slot_of_node] = x0

    conv_w = np.stack([np.asarray(w, dtype=np.float32) for w in conv_ws])  # [L,B,F,F]
    fc_w = np.stack([np.asarray(w, dtype=np.float32) for w in fc_ws])      # [L,F,F]
    fc_b = np.stack([np.asarray(b, dtype=np.float32) for b in fc_bs])      # [L,F]

    # ------------------------------ build graph ------------------------------
    nc = bacc.Bacc("TRN2", target_bir_lowering=False, debug=False, num_devices=NCORE,
                   num_swdge_queues=4)

    d_xq0 = nc.dram_tensor("xq0", [GE, NG * F], bf16, kind="ExternalInput")
    d_x0T = nc.dram_tensor("x0T", [128, NU], bf16, kind="ExternalInput")
    d_sphi = nc.dram_tensor("sphi", [GE, NG * GCOL], bf16, kind="ExternalInput")
    d_gidx = nc.dram_tensor("gidx", [128, NG * GE // 16], i16, kind="ExternalInput")
    d_convw = nc.dram_tensor("convw", [128, N_LAYERS * B * F], bf16,
                             kind="ExternalInput")
    d_fcw = nc.dram_tensor("fcw", [128, N_LAYERS * F], bf16, kind="ExternalInput")
    d_fcb = nc.dram_tensor("fcb", [128, N_LAYERS], f32, kind="ExternalInput")
    d_out = nc.dram_tensor("out", [NSLOT, F], f32, kind="ExternalOutput")
    d_ash = [nc.dram_tensor(f"a_shard{h}", [s1 - s0, F], bf16, kind="Internal")
             for h, (s0, s1) in enumerate(parts)]
    d_afull = [nc.dram_tensor(f"a_full{h}", [NCORE * (s1 - s0), F], bf16,
                              kind="Internal", addr_space="Shared")
               for h, (s0, s1) in enumerate(parts)]
    d_apad = nc.dram_tensor("a_pad", [NROWS, EP], bf16, kind="Internal")
    d_wu_in = nc.dram_tensor("wu_in", [128, 16], bf16, kind="Internal")
    d_wu_out = nc.dram_tensor("wu_out", [NCORE * 128, 16], bf16, kind="Internal",
                              addr_space="Shared")

    with tile.TileContext(nc) as tc:
        with (
            tc.tile_pool(name="persist", bufs=1) as pp,
            tc.tile_pool(name="sphi", bufs=3) as sp,
            tc.tile_pool(name="xj", bufs=10) as xp,
            tc.tile_pool(name="work", bufs=3) as wp,
            tc.tile_pool(name="psc", bufs=3, space="PSUM") as psc,
            tc.tile_pool(name="pout", bufs=2, space="PSUM") as pout,
            tc.tile_pool(name="ptr", bufs=1, space="PSUM") as ptr,
        ):
            nc.gpsimd.load_library(library_config.mlp)

            # tiny warmup collective: absorbs first-collective link setup cost
            # while layer-0 compute runs
            nc.gpsimd.collective_compute(
                "AllGather", mybir.AluOpType.bypass,
                replica_groups=[list(range(NCORE))],
                ins=[d_wu_in[:]], outs=[d_wu_out[:]])

            gidx_sb = pp.tile([128, NG * GE // 16], i16)
            nc.sync.dma_start(out=gidx_sb[:], in_=d_gidx[:])
            convw_sb = pp.tile([128, N_LAYERS * B * F], bf16)
            nc.sync.dma_start(out=convw_sb[:], in_=d_convw[:])
            fcw_sb = pp.tile([128, N_LAYERS * F], bf16)
            nc.sync.dma_start(out=fcw_sb[:], in_=d_fcw[:])
            fcb_sb = pp.tile([128, N_LAYERS], f32)
            nc.sync.dma_start(out=fcb_sb[:], in_=d_fcb[:])
            id_bf = pp.tile([128, F], bf16)
            id_f32 = pp.tile([128, F], f32)
            for j in range(4):
                make_identity(nc, id_bf[32 * j:32 * j + 32, :])
                make_identity(nc, id_f32[32 * j:32 * j + 32, :])
            # strip-stacked state: partition 32j+c <-> (group strip j, feature c)
            # column p*GCOL + s*16 + b <-> (pack p, slot s, basis b)
            m_stk = pp.tile([128, NPACK * GCOL], bf16)
            ansT = pp.tile([128, NU], f32)
            aT = pp.tile([128, NU], bf16)
            nc.sync.dma_start(out=aT[:], in_=d_x0T[:])
            xq0_sb = pp.tile([GE, NG * F], bf16)      # layer-0 x_j, host pre-gathered
            nc.sync.dma_start(out=xq0_sb[:], in_=d_xq0[:])

            for l in range(N_LAYERS):
                if l > 0:
                    for h, (t0, t1) in enumerate(tab_ranges):
                        nc.gpsimd.collective_compute(
                            "AllGather", mybir.AluOpType.bypass,
                            replica_groups=[list(range(NCORE))],
                            ins=[d_ash[h][:]], outs=[d_afull[h][:]])
                        nc.sync.dma_start(out=d_apad[t0:t1, 0:F],
                                          in_=d_afull[h][:])

                # ---------- gather + scatter-GEMM, chunked by GCHUNK groups ----------
                NIC = GCHUNK * GE             # gather idxs per chunk
                for ch in range(NCHUNK):
                    if l == 0:
                        xq = xq0_sb
                        xofs, xstride = ch * GCHUNK * F, F
                    else:
                        xq = xp.tile([GE, GCHUNK * EP], bf16, tag="xj")
                        xofs, xstride = 0, EP
                        nc.gpsimd.dma_gather(
                            out_ap=xq[:].rearrange("p (g e) -> p g e", e=EP),
                            in_ap=d_apad[:],
                            idxs_ap=gidx_sb[:, ch * NIC // 16:(ch + 1) * NIC // 16],
                            num_idxs=NIC, num_idxs_reg=NIC, elem_size=EP,
                            queue_num=ch % 4)
                    st = sp.tile([GE, GCHUNK * GCOL], bf16, tag="st")
                    nc.sync.dma_start(
                        out=st[:],
                        in_=d_sphi[:, ch * GCHUNK * GCOL:(ch + 1) * GCHUNK * GCOL])
                    for g4 in range(GCHUNK // 4):
                        # 4 groups packed into the 4 PE column-groups: the
                        # matmuls run concurrently (separate XBUS streams) and
                        # land in 4 partition strips of one PSUM tile
                        ps = psc.tile([128, GCOL], f32, tag="psc")
                        for j in range(4):
                            gl = g4 * 4 + j
                            nc.tensor.matmul(
                                out=ps[32 * j:32 * j + 32, :],
                                lhsT=xq[:, xofs + gl * xstride:xofs + gl * xstride + F],
                                rhs=st[:, gl * GCOL:(gl + 1) * GCOL],
                                start=True, stop=True, tile_position=(0, 32 * j))
                        pk = (ch * GCHUNK + g4 * 4) // 4
                        dst = m_stk[:, pk * GCOL:(pk + 1) * GCOL]
                        if pk % 2 == 0:
                            nc.scalar.copy(out=dst, in_=ps[:])
                        else:
                            nc.vector.tensor_copy(out=dst, in_=ps[:])

                # ---------- node-side GEMM + epilogue per batch (strip space) ----
                if DEBUG_STAGE == 1:
                    continue
                for bi, w in enumerate(widths):
                    u0, nU1 = boffs[bi] // 4, w // 4
                    U = slice(u0, u0 + nU1)
                    p0, p1 = boffs[bi] // 32, (boffs[bi] + w) // 32
                    po = pout.tile([128, nU1], f32, tag="pout")
                    for b in range(B):
                        wofs = (l * B + b) * F
                        for j in range(4):
                            rhs = m_stk[32 * j:32 * j + 32,
                                        p0 * GCOL:p1 * GCOL].rearrange(
                                "c (p s b) -> c b (p s)", s=GS, b=B)[:, b:b + 1, :]
                            nc.tensor.matmul(
                                out=po[32 * j:32 * j + 32, :],
                                lhsT=convw_sb[32 * j:32 * j + 32, wofs:wofs + F],
                                rhs=rhs, start=(b == 0), stop=False,
                                tile_position=(32 * j, 32 * j))
                    for j in range(4):
                        nc.tensor.matmul(
                            out=po[32 * j:32 * j + 32, :],
                            lhsT=fcw_sb[32 * j:32 * j + 32, l * F:(l + 1) * F],
                            rhs=aT[32 * j:32 * j + 32, U], start=False, stop=True,
                            tile_position=(32 * j, 32 * j))

                    if DEBUG_STAGE == 2:
                        continue
                    if l == 0:
                        nc.scalar.activation(
                            out=ansT[:, U], in_=po[:],
                            func=mybir.ActivationFunctionType.Identity,
                            bias=fcb_sb[:, l:l + 1])
                    else:
                        tmp = wp.tile([128, nU1], f32, tag="tmp")
                        nc.scalar.activation(
                            out=tmp[:], in_=po[:],
                            func=mybir.ActivationFunctionType.Identity,
                            bias=fcb_sb[:, l:l + 1])
                        nc.vector.tensor_add(
                            out=ansT[:, U], in0=ansT[:, U], in1=tmp[:])

                    if l < N_LAYERS - 1:
                        nc.scalar.activation(
                            out=aT[:, U], in_=ansT[:, U],
                            func=mybir.ActivationFunctionType.Relu)
                        aout = wp.tile([128, 4 * F], bf16, tag="aout")
                        for j in range(4):
                            pt = ptr.tile([128, F], bf16, tag="ptrb")
                            nc.tensor.transpose(out=pt[0:nU1, :],
                                                in_=aT[32 * j:32 * j + 32, U],
                                                identity=id_bf[32 * j:32 * j + 32, :],
                                                tile_position=(32 * j, 0))
                            nc.vector.tensor_copy(
                                out=aout[0:nU1, j * F:(j + 1) * F], in_=pt[0:nU1, :])
                        nc.sync.dma_start(
                            out=d_ash[bi][:].rearrange("(u j) f -> u j f", j=4),
                            in_=aout[0:nU1, :].rearrange("p (j f) -> p j f", f=F))
                    else:
                        oT = wp.tile([128, nU1], f32, tag="oT")
                        nc.scalar.activation(
                            out=oT[:], in_=ansT[:, U],
                            func=mybir.ActivationFunctionType.Copy,
                            scale=OUT_SCALE)
                        oout = wp.tile([128, 4 * F], f32, tag="oout")
                        for j in range(4):
                            pt = ptr.tile([128, F], f32, tag="ptr")
                            nc.tensor.transpose(out=pt[0:nU1, :],
                                                in_=oT[32 * j:32 * j + 32, :],
                                                identity=id_f32[32 * j:32 * j + 32, :],
                                                tile_position=(32 * j, 0))
                            nc.vector.tensor_copy(
                                out=oout[0:nU1, j * F:(j + 1) * F], in_=pt[0:nU1, :])
                        nc.sync.dma_start(
                            out=d_out[boffs[bi]:boffs[bi] + w, :].rearrange(
                                "(u j) f -> u j f", j=4),
                            in_=oout[0:nU1, :].rearrange("p (j f) -> p j f", f=F))

    nc.compile()

    # ------------------------------ run ------------------------------
    convw_c = np.ascontiguousarray(
        conv_w.transpose(2, 0, 1, 3).reshape(F, N_LAYERS * B * F)
    ).astype(ml_dtypes.bfloat16)
    convw_stk = np.ascontiguousarray(np.tile(convw_c, (4, 1)))
    fcw_c = np.ascontiguousarray(
        fc_w.transpose(1, 0, 2).reshape(F, N_LAYERS * F)).astype(ml_dtypes.bfloat16)
    fcw_stk = np.ascontiguousarray(np.tile(fcw_c, (4, 1)))
    fcb_stk = np.ascontiguousarray(np.tile(fc_b.T, (4, 1)))
    sphi_bf = sphi.astype(ml_dtypes.bfloat16)
    x0_slots_bf = x0_slots.astype(ml_dtypes.bfloat16)

    in_maps = []
    for c in range(NCORE):
        # strip-stacked transposed x0: [32j+f, p*GS+s] = x0[slot (4p+j, s)]
        x0_c = x0_slots[c * NSLOT:(c + 1) * NSLOT]
        x0T_c = np.ascontiguousarray(
            x0_c.reshape(NPACK, 4, GS, F).transpose(1, 3, 0, 2)
            .reshape(128, NU)).astype(ml_dtypes.bfloat16)
        sphi_c = np.ascontiguousarray(
            sphi_bf[c].transpose(1, 0, 2).reshape(GE, NG * GCOL))
        # layer-0 x_j gathered on host: xq0[p, gi*F:(gi+1)*F] = x0[gsrc[c][p, gi]]
        xq0_c = np.ascontiguousarray(
            x0_slots_bf[gsrc[c].T.astype(np.int64)].transpose(1, 0, 2)
            .reshape(GE, NG * F))
        in_maps.append({
            "xq0": xq0_c,
            "x0T": x0T_c,
            "sphi": sphi_c,
            "gidx": np.ascontiguousarray(idx_w[c]),
            "convw": convw_stk,
            "fcw": fcw_stk,
            "fcb": fcb_stk,
        })

    res = run_bass_kernel_spmd(nc, in_maps, core_ids=list(range(NCORE)),
                               trace=PROFILE)
    global LAST_EXEC_NS, LAST_TRACE
    LAST_EXEC_NS = res.exec_time_ns
    LAST_TRACE = res.instructions_and_trace[1] if res.instructions_and_trace else None
    out_slots = np.concatenate([res.results[c]["out"] for c in range(NCORE)], axis=0)
    # per-core output rows are in strip-stacked (u*4 + j) order
    outrow_cat = (np.arange(NCORE)[:, None] * NSLOT + outrow_of_slot[None, :]).reshape(-1)
    return out_slots[outrow_cat[slot_of_node]].astype(np.float32)


def kernel(fluidFeatures, edge_i, edge_j, edge_attr, conv_ws, fc_ws, fc_bs):
    pre = _preprocess(edge_i, edge_j, edge_attr)
    return _build_and_run(pre, fluidFeatures, conv_ws, fc_ws, fc_bs)

